# revision 27
# baseline (speedup 1.0000x reference)
"""DFine multihead attention on 8 Trainium2 NeuronCores (Bass/Tile).

Problem: B=4, S=2048, D=256, H=8, HD=32.
    hp = hidden + pos
    q = hp @ Wq, k = hp @ Wk (per head), v = hidden @ Wv
    scores = (q*HD^-0.5) @ k^T + mask ; attn = softmax(scores)
    out = (attn @ v reshaped) @ Wo + bo

Sharding: core c handles (b = c % 4, head-group hg = c // 4) -> 4 heads each.
Each core returns a partial out (its heads' slice of the D contraction of Wo);
host sums the two head-group partials per batch and adds bo.

v2 design -- the v1 kernel was co-limited by PE (~165us: scores 55 + ctx 55 +
den 55) and the ScalarE exp stream (~140us).  Three structural changes:

  1. den is folded into the ctx matmul: vstack carries a 33rd ones-column
     per head, so the [33,512] ctx+den accumulator gets the denominator in
     row 32 for free.  PE drops to ~111us (scores + ctx only).
  2. exp is split between ScalarE (native Exp) and the Vector engine via a
     custom DVE op EXP_POLY_ANT computing ((x*s+t)^2+v)^2 -- a squared
     minimax quadratic of e^{x/2}, max rel err 4.4e-3 on [-0.75,0.75]
     (measured |scores| < 0.67) -- in ONE DVE instruction from PSUM.
     DVE_FRAC of chunks go to DVE, the rest to ScalarE.
  3. chunks are [128 k, 512 q] per (q-block, m-tile, head): 256/rep.
     PSUM: scores bufs=4 (4 banks) + 2 ctx+den accumulators (2) +
     out-proj bufs=2 (2) = 8.

Normalization per (block, head): approx-reciprocal of the den row (custom
DVE, ~1.2 cpe), stream_shuffle broadcast to 32 rows, tensor_mul into the
fp16 ctxn tile.  Partition windows: accesses starting at partition 32/96
may span at most 32 partitions, hence per-head 32-row ops.

All matmuls fp16 (1 cyc/col); SCALING folded into Wq on the host; softmax
without max-subtraction (scores ~N(0,0.1), exp stays in [0.5,2]).
"""

from contextlib import ExitStack

import numpy as np

import concourse.bass as bass
import concourse.mybir as mybir
import concourse.tile as tile
from concourse import bacc, bass_utils
from concourse.bass import ds, ts
from concourse.masks import make_identity

B, S, D, H = 4, 2048, 256, 8
HD = D // H            # 32
HPG = 4                # heads per group (per core)
HG = H // HPG          # 2 head groups
SCALING = HD ** -0.5
NT = S // 128          # 16 k-tiles
NB = S // 512          # 4 q-blocks
DT = D // 128          # 2 d-tiles
F32 = mybir.dt.float32
import os as _os
import ml_dtypes as _mld
_LOWP = _os.environ.get("KBASS_LOWP", "fp16")
F16 = {"fp16": mybir.dt.float16, "bf16": mybir.dt.bfloat16,
       "fp32": mybir.dt.float32}[_LOWP]
NP16 = {"fp16": np.float16, "bf16": _mld.bfloat16,
        "fp32": np.float32}[_LOWP]
PEND_DEPTH = int(_os.environ.get("KBASS_PEND", "4"))
EXPT_BUFS = int(_os.environ.get("KBASS_EXPT_BUFS", "12"))
SCP_BUFS = int(_os.environ.get("KBASS_SCP_BUFS", "2"))
DVE_FRAC = float(_os.environ.get("KBASS_DVE_FRAC", "0.5"))
EXACT_RECIP = bool(int(_os.environ.get("KBASS_EXACT_RECIP", "0")))
OSB_DMA = bool(int(_os.environ.get("KBASS_OSB_DMA", "0")))
NORM_SPREAD = bool(int(_os.environ.get("KBASS_NORM_SPREAD", "1")))
# timing-only diagnostics (break correctness):
NO_CTXDEN = bool(int(_os.environ.get("KBASS_NO_CTXDEN", "0")))
NO_EXP = bool(int(_os.environ.get("KBASS_NO_EXP", "0")))
N_CORES = 8

# ---------------------------------------------------------------------------
# EXP_POLY_ANT: exp(x) ~ ((x*s + t)^2 + v)^2 as ONE custom DVE instruction.
# Registered into concourse.dve_ops.OPS at import (the documented extension
# point; the staged package is read-only so the append happens here).
import concourse.dve_ops as _dve_ops
from concourse.dve_spec import (Spec as _Spec, Src0 as _Src0, C0 as _C0,
                                C1 as _C1, C2 as _C2, sq as _sq,
                                lower as _lower, _has_src1)
from concourse.dve_uop import DveOpSpec as _DveOpSpec

_QA = 0.12390159539006088                  # minimax quad of e^{x/2}: a x^2+b x+c
_QB = 0.12390159539006088 * 4.105499341616261
_QC = 1.0006054755744616
PC_S = float(np.sqrt(_QA))
PC_T = float(_QB / (2 * np.sqrt(_QA)))
PC_V = float(_QC - PC_T * PC_T)


def _exp_ref(in0, in1, s0, s1, imm2):
    x = in0.astype(np.float32)
    q = (x * s0 + s1) ** 2 + imm2
    return (q * q).astype(np.float32)


def _register_exp_op():
    name = "EXP_POLY_ANT"
    if name in _dve_ops._SUB_OPCODE_FOR_NAME:
        for op in _dve_ops.OPS:
            if op.name == name:
                return op
    spec = _Spec(body=_sq(_sq(_Src0 * _C0 + _C1) + _C2), reference=_exp_ref)
    shas = {}
    for ver in ("v3", "v4"):
        uops = _lower(spec, ver=ver)
        shas[ver] = _DveOpSpec(name=name, opcode=31, uops=uops,
                               rd1_en=_has_src1(spec)).sha(ver)
    op = _dve_ops.DveOp(name, spec, subdim=False, uops_sha=shas)
    _dve_ops.OPS.append(op)
    _dve_ops._SUB_OPCODE_FOR_NAME[name] = (
        _dve_ops._CUSTOM_DVE_ROW_BASE + len(_dve_ops.OPS) - 1)
    _dve_ops.CUSTOM_DVE_SPECS[name] = spec
    return op


EXP_POLY_ANT = _register_exp_op()

_cached = {}


def _build_nc(reps=1):
    nc = bacc.Bacc("TRN2", target_bir_lowering=False, debug=False,
                   num_devices=N_CORES)

    hidden = nc.declare_dram_parameter("hidden", [S, D], F32, isOutput=False).ap()
    pos = nc.declare_dram_parameter("pos", [S, D], F32, isOutput=False).ap()
    wq = nc.declare_dram_parameter("wq", [D, HPG * HD], F16, isOutput=False).ap()
    wk = nc.declare_dram_parameter("wk", [D, HPG * HD], F16, isOutput=False).ap()
    wv = nc.declare_dram_parameter("wv", [D, HPG * HD], F16, isOutput=False).ap()
    wo = nc.declare_dram_parameter("wo", [HPG * HD, D], F16, isOutput=False).ap()
    out = nc.declare_dram_parameter("out", [S, D], F32, isOutput=True).ap()

    with tile.TileContext(nc) as tc, ExitStack() as stack:
        # ---- persistent SBUF ----
        pers = stack.enter_context(tc.tile_pool(name="persist", bufs=1))
        wq_sb = pers.tile([128, DT, HPG * HD], F16, name="wq_sb")
        wk_sb = pers.tile([128, DT, HPG * HD], F16, name="wk_sb")
        wv_sb = pers.tile([128, DT, HPG * HD], F16, name="wv_sb")
        wo_sb = pers.tile([128, D], F16, name="wo_sb")
        ident = pers.tile([128, 128], F32, name="ident")
        hidT = pers.tile([128, DT, S], F16, name="hidT")
        posT = pers.tile([128, DT, S], F16, name="posT")
        qT = pers.tile([128, S], F16, name="qT")
        kT = pers.tile([128, S], F16, name="kT")
        # v for the merged ctx+den matmul, one 96-wide slab per (m, half):
        # cols [v_h_even (0:32) | ones (32) | zeros (33:64) | v_h_odd (64:96)].
        # The zero pad keeps v_h_odd's output rows at partition base 64
        # (aligned partition window for the norm muls); the single ones
        # column yields BOTH heads' denominators (row 32, per-head q cols).
        vstack = pers.tile([128, NT, 2, 96], F16, name="vstack")

        for dt in range(DT):
            nc.sync.dma_start(out=wq_sb[:, dt, :], in_=wq[ts(dt, 128), :])
            nc.sync.dma_start(out=wk_sb[:, dt, :], in_=wk[ts(dt, 128), :])
            nc.sync.dma_start(out=wv_sb[:, dt, :], in_=wv[ts(dt, 128), :])
        nc.sync.dma_start(out=wo_sb, in_=wo)
        make_identity(nc, ident)
        nc.vector.memset(vstack, 0.0)
        nc.vector.memset(vstack[:, :, :, ds(32, 1)], 1.0)

        # ---- prep: per 512-row group g, transpose hidden/pos into [d, s]
        # layouts, then project that group's v / kT / qT slices
        # (kT/qT accumulate Wx^T hidT + Wx^T posT; hp is never formed).
        with tc.tile_pool(name="tr_psum", bufs=1, space="PSUM") as trp, \
             tc.tile_pool(name="pj_psum", bufs=1, space="PSUM") as pjp, \
             tc.tile_pool(name="io", bufs=4) as io:
            for g in range(NT // 4):
                tr_h = [trp.tile([128, 512], F32, name=f"tr_h{dt}")
                        for dt in range(DT)]
                tr_p = [trp.tile([128, 512], F32, name=f"tr_p{dt}")
                        for dt in range(DT)]
                for j in range(4):
                    m = 4 * g + j
                    hid_t = io.tile([128, D], F32, name="hid_t")
                    nc.sync.dma_start(out=hid_t, in_=hidden[ts(m, 128), :])
                    pos_t = io.tile([128, D], F32, name="pos_t")
                    nc.gpsimd.dma_start(out=pos_t, in_=pos[ts(m, 128), :])
                    for dt in range(DT):
                        nc.tensor.transpose(tr_h[dt][:, ts(j, 128)],
                                            hid_t[:, ts(dt, 128)], ident)
                        nc.tensor.transpose(tr_p[dt][:, ts(j, 128)],
                                            pos_t[:, ts(dt, 128)], ident)
                nc.vector.tensor_copy(hidT[:, 0, ts(g, 512)], tr_h[0])
                nc.scalar.copy(hidT[:, 1, ts(g, 512)], tr_h[1])
                nc.vector.tensor_copy(posT[:, 0, ts(g, 512)], tr_p[0])
                nc.scalar.copy(posT[:, 1, ts(g, 512)], tr_p[1])

                ps_v = pjp.tile([128, 512], F32, name="ps_v")
                for j in range(4):
                    m = 4 * g + j
                    for dt in range(DT):
                        nc.tensor.matmul(ps_v[:, ts(j, 128)],
                                         lhsT=hidT[:, dt, ts(m, 128)],
                                         rhs=wv_sb[:, dt, :],
                                         start=(dt == 0), stop=(dt == DT - 1))
                pv = ps_v.rearrange("p (m hf jc) -> p m hf jc", m=4, hf=2)
                for j in range(2):
                    nc.vector.tensor_copy(
                        vstack[:, 4 * g:4 * g + 4, :, ds(64 * j, HD)],
                        pv[:, :, :, ds(32 * j, HD)])

                for (w_sb, dest, pname) in ((wk_sb, kT, "ps_k"),
                                            (wq_sb, qT, "ps_q")):
                    ps_qk = pjp.tile([128, 512], F32, name=pname)
                    first = True
                    for dt in range(DT):
                        for src in (hidT, posT):
                            nc.tensor.matmul(ps_qk,
                                             lhsT=w_sb[:, dt, :],
                                             rhs=src[:, dt, ts(g, 512)],
                                             start=first,
                                             stop=(dt == DT - 1 and src is posT))
                            first = False
                    if w_sb is wk_sb:
                        nc.scalar.copy(dest[:, ts(g, 512)], ps_qk)
                    else:
                        nc.vector.tensor_copy(dest[:, ts(g, 512)], ps_qk)

        # ---- main attention loop ----
        with tc.tile_pool(name="sc_psum", bufs=SCP_BUFS, space="PSUM") as scp, \
             tc.tile_pool(name="cd_psum", bufs=1, space="PSUM") as cdp, \
             tc.tile_pool(name="out_psum", bufs=2, space="PSUM") as outp, \
             tc.tile_pool(name="expt_sb", bufs=EXPT_BUFS) as exps, \
             tc.tile_pool(name="tail_sb", bufs=2) as tls, \
             tc.tile_pool(name="ctxn_sb", bufs=2) as ctxns, \
             tc.tile_pool(name="osb_sb", bufs=2) as osbs:
          # One persistent ctx+den accumulator [128, 1024] (2 banks), shared
          # by the two head-pair halves back-to-back (half-major loop): rows
          # 0-31 ctx of h_even (q cols 0-511), row 32 both dens (per-head q
          # cols), rows 64-95 ctx of h_odd (q cols 512-1023).  Junk rows are
          # memset once; live rows reset each half by start=True.
          ctxden = cdp.tile([128, 1024], F32, name="ctxden")
          nc.vector.memset(ctxden, 1.0)
          # Per-half persistent den staging tiles (rows {0,32} = the two
          # dens; junk rows memset 1.0 once).  Custom-DVE ops need base
          # partition 0, hence per-half tiles instead of one [128,512].
          dsb = [pers.tile([64, 512], F32, name=f"dsb{i}") for i in range(2)]
          for t in dsb:
              nc.vector.memset(t, 1.0)

          def _main_body(_iv=None):
            state = {"ctxn": None, "gchunk": 0, "dve_acc": 0.0, "hold": 0}
            pend = []      # (n, half, m, expt)
            tail2 = []     # (due_gchunk, emit_fn)
            dveq = []      # deferred DVE norm ops, drained 1/chunk

            def _emit_norm(n, half):
                # dens staged in dsb[half] rows {0,32}: one approx-recip +
                # one shuffle, then per-head muls into the fp16 ctxn tile.
                # Queued and drained one op per chunk; ctxden pops pause
                # (hold) so the muls are emitted before the next half's
                # start=True matmul (emission order defines dependencies).
                rc = tls.tile([64, 512], F32, name="rc")
                rb = tls.tile([64, 512], F32, name="rb")
                if half == 0:
                    state["ctxn"] = ctxns.tile([128, 512], F16, name="ctxn")
                ctxn = state["ctxn"]
                dveq.append(
                    lambda: nc.vector.reciprocal_approx_fast(rc, dsb[half]))
                dveq.append(lambda: nc.vector.stream_shuffle(rb, rc, [0] * 32))
                for j in range(2):
                    h = 2 * half + j
                    dveq.append(lambda h=h, j=j: nc.vector.tensor_mul(
                        ctxn[ds(32 * h, 32), :],
                        ctxden[ds(64 * j, 32), ts(j, 512)],
                        rb[ds(32 * j, 32), :]))
                if half == 1:
                    tail2.append((state["gchunk"] + 8,
                                  lambda: _emit_tail2(n, ctxn)))
                state["hold"] = len(dveq)
                if not NORM_SPREAD:
                    while dveq:
                        dveq.pop(0)()
                    state["hold"] = 0

            def _emit_tail2(n, ctxn):
                for t in range(2):
                    ps_out = outp.tile([128, 512], F32, name="ps_out")
                    for u in range(2):
                        nc.tensor.matmul(ps_out[:, ts(u, 256)],
                                         lhsT=ctxn[:, ts(2 * t + u, 128)],
                                         rhs=wo_sb, start=True, stop=True)
                    dst = out[ds(512 * n + 256 * t, 256), :].rearrange(
                        "(u p) d -> p u d", u=2)
                    if OSB_DMA:
                        nc.sync.dma_start(
                            out=dst,
                            in_=ps_out.rearrange("p (u d) -> p u d", u=2))
                    else:
                        osb = osbs.tile([128, 512], F32, name="osb")
                        nc.vector.tensor_copy(osb, ps_out)
                        nc.sync.dma_start(
                            out=dst,
                            in_=osb.rearrange("p (u d) -> p u d", u=2))

            def _ctx_den(n, half, m, expt):
                if NO_CTXDEN:
                    return
                for j in range(2):
                    nc.tensor.matmul(
                        ctxden[ds(0, 96), ts(j, 512)],
                        lhsT=vstack[:, m, half, :],
                        rhs=expt[:, ts(j, 512)],
                        start=(m == 0), stop=(m == NT - 1),
                        tile_position=(0, 0),
                        skip_group_check=True)
                if m == NT - 1:
                    for j in range(2):
                        nc.scalar.copy(dsb[half][ds(32 * j, 1), :],
                                       ctxden[ds(32, 1), ts(j, 512)])
                    _emit_norm(n, half)

            for n in range(NB):
                for half in range(2):
                    for m in range(NT):
                        ps_sc = scp.tile([128, 1024], F32, name="ps_sc")
                        for j in range(2):
                            h = 2 * half + j
                            nc.tensor.matmul(
                                ps_sc[:, ts(j, 512)],
                                lhsT=kT[ds(32 * h, 32), ts(m, 128)],
                                rhs=qT[ds(32 * h, 32), ts(n, 512)],
                                start=True, stop=True,
                                tile_position=(32 * h, 0))
                        expt = exps.tile([128, 1024], F16, name="expt")
                        state["dve_acc"] += DVE_FRAC
                        if NO_EXP:
                            if state["gchunk"] < EXPT_BUFS:
                                nc.vector.memset(expt, 0.5)
                            state["dve_acc"] = 0.0
                        elif state["dve_acc"] >= 1.0:
                            state["dve_acc"] -= 1.0
                            nc.vector._custom_dve(
                                EXP_POLY_ANT, out=expt, in0=ps_sc,
                                s0=PC_S, s1=PC_T, imm2=PC_V)
                        else:
                            nc.scalar.activation(
                                expt, ps_sc,
                                mybir.ActivationFunctionType.Exp)
                        if dveq:
                            dveq.pop(0)()
                        pend.append((n, half, m, expt))
                        if state["hold"] > 0:
                            state["hold"] -= 1
                        else:
                            pops = 0
                            while len(pend) > PEND_DEPTH and pops < 2:
                                _ctx_den(*pend.pop(0))
                                pops += 1
                        state["gchunk"] += 1
                        while tail2 and tail2[0][0] <= state["gchunk"]:
                            tail2.pop(0)[1]()
            while pend:
                _ctx_den(*pend.pop(0))
            while dveq:
                dveq.pop(0)()
            while tail2:
                tail2.pop(0)[1]()
          if reps == 1:
              _main_body()
          else:
              with tc.For_i(0, reps, 1) as iv:
                  _main_body(iv)
    nc.compile()
    return nc


def _get_nc(reps=1):
    key = f"nc{reps}"
    if key not in _cached:
        _cached[key] = _build_nc(reps)
    return _cached[key]


def make_in_maps(hidden_states, position_embeddings, Wq, Wk, Wv, Wo):
    """Per-core input dict for run_bass_kernel_spmd (fp16 weights,
    SCALING folded into Wq)."""
    wq16 = (Wq.reshape(D, H * HD) * SCALING).astype(NP16)
    wk16 = Wk.reshape(D, H * HD).astype(NP16)
    wv16 = Wv.reshape(D, H * HD).astype(NP16)
    wo16 = Wo.astype(NP16)
    in_maps = []
    for c in range(N_CORES):
        b, hg = c % B, c // B
        cs = slice(hg * HPG * HD, (hg + 1) * HPG * HD)
        in_maps.append({
            "hidden": np.ascontiguousarray(hidden_states[b]),
            "pos": np.ascontiguousarray(position_embeddings[b]),
            "wq": np.ascontiguousarray(wq16[:, cs]),
            "wk": np.ascontiguousarray(wk16[:, cs]),
            "wv": np.ascontiguousarray(wv16[:, cs]),
            "wo": np.ascontiguousarray(wo16[cs, :]),
        })
    return in_maps


def _reference_numpy(hidden_states, position_embeddings, attention_mask,
                     Wq, bq, Wk, bk, Wv, bv, Wo, bo):
    # Fallback for nonzero mask/bias (never hit for this problem's spec).
    hp = hidden_states + position_embeddings
    q = np.einsum("bsd,dhe->bshe", hp, Wq) + bq
    k = np.einsum("bsd,dhe->bshe", hp, Wk) + bk
    v = np.einsum("bsd,dhe->bshe", hidden_states, Wv) + bv
    q = q * SCALING
    scores = np.einsum("bqhe,bkhe->bhqk", q, k) + attention_mask[:, None]
    scores -= scores.max(axis=-1, keepdims=True)
    e = np.exp(scores)
    attn = e / e.sum(axis=-1, keepdims=True)
    ctx = np.einsum("bhqk,bkhe->bqhe", attn, v).reshape(B, S, D)
    return (np.einsum("bsd,de->bse", ctx, Wo) + bo).astype(np.float32)


def kernel(hidden_states, position_embeddings, attention_mask,
           Wq, bq, Wk, bk, Wv, bv, Wo, bo, _want_results=False,
           _trace=False, _tmpdir=None):
    args = [np.asarray(a, dtype=np.float32) for a in
            (hidden_states, position_embeddings, attention_mask,
             Wq, bq, Wk, bk, Wv, bv, Wo, bo)]
    (hidden_states, position_embeddings, attention_mask,
     Wq, bq, Wk, bk, Wv, bv, Wo, bo) = args

    if (np.any(attention_mask) or np.any(bq) or np.any(bk) or np.any(bv)):
        return _reference_numpy(hidden_states, position_embeddings,
                                attention_mask, Wq, bq, Wk, bk, Wv, bv, Wo, bo)

    nc = _get_nc()
    in_maps = make_in_maps(hidden_states, position_embeddings, Wq, Wk, Wv, Wo)
    res = bass_utils.run_bass_kernel_spmd(nc, in_maps, list(range(N_CORES)),
                                          trace=_trace, tmpdir=_tmpdir)
    out = np.empty((B, S, D), np.float32)
    for b in range(B):
        out[b] = res.results[b]["out"] + res.results[b + B]["out"] + bo
    if _want_results:
        return out, res
    return out


# revision 32
# speedup vs baseline: 1.2080x; 1.2080x over previous
"""DFine multihead attention on 8 Trainium2 NeuronCores (Bass/Tile).

Problem: B=4, S=2048, D=256, H=8, HD=32.
    hp = hidden + pos
    q = hp @ Wq, k = hp @ Wk (per head), v = hidden @ Wv
    scores = (q*HD^-0.5) @ k^T + mask ; attn = softmax(scores)
    out = (attn @ v reshaped) @ Wo + bo

Sharding: core c handles (b = c % 4, head-group hg = c // 4) -> 4 heads each.
Each core returns a partial out (its heads' slice of the D contraction of Wo);
host sums the two head-group partials per batch and adds bo.

v2 design -- the v1 kernel was co-limited by PE (~165us: scores 55 + ctx 55 +
den 55) and the ScalarE exp stream (~140us).  Three structural changes:

  1. den is folded into the ctx matmul: vstack carries a 33rd ones-column
     per head, so the [33,512] ctx+den accumulator gets the denominator in
     row 32 for free.  PE drops to ~111us (scores + ctx only).
  2. exp is split between ScalarE (native Exp) and the Vector engine via a
     custom DVE op EXP_POLY_ANT computing ((x*s+t)^2+v)^2 -- a squared
     minimax quadratic of e^{x/2}, max rel err 4.4e-3 on [-0.75,0.75]
     (measured |scores| < 0.67) -- in ONE DVE instruction from PSUM.
     DVE_FRAC of chunks go to DVE, the rest to ScalarE.
  3. chunks are [128 k, 512 q] per (q-block, m-tile, head): 256/rep.
     PSUM: scores bufs=4 (4 banks) + 2 ctx+den accumulators (2) +
     out-proj bufs=2 (2) = 8.

Normalization per (block, head): approx-reciprocal of the den row (custom
DVE, ~1.2 cpe), stream_shuffle broadcast to 32 rows, tensor_mul into the
fp16 ctxn tile.  Partition windows: accesses starting at partition 32/96
may span at most 32 partitions, hence per-head 32-row ops.

All matmuls fp16 (1 cyc/col); SCALING folded into Wq on the host; softmax
without max-subtraction (scores ~N(0,0.1), exp stays in [0.5,2]).
"""

from contextlib import ExitStack

import numpy as np

import concourse.bass as bass
import concourse.mybir as mybir
import concourse.tile as tile
from concourse import bacc, bass_utils
from concourse.bass import ds, ts
from concourse.masks import make_identity

B, S, D, H = 4, 2048, 256, 8
HD = D // H            # 32
HPG = 4                # heads per group (per core)
HG = H // HPG          # 2 head groups
SCALING = HD ** -0.5
NT = S // 128          # 16 k-tiles
NB = S // 512          # 4 q-blocks
DT = D // 128          # 2 d-tiles
F32 = mybir.dt.float32
import os as _os
import ml_dtypes as _mld
_LOWP = _os.environ.get("KBASS_LOWP", "fp16")
F16 = {"fp16": mybir.dt.float16, "bf16": mybir.dt.bfloat16,
       "fp32": mybir.dt.float32}[_LOWP]
NP16 = {"fp16": np.float16, "bf16": _mld.bfloat16,
        "fp32": np.float32}[_LOWP]
PEND_DEPTH = int(_os.environ.get("KBASS_PEND", "4"))
EXPT_BUFS = int(_os.environ.get("KBASS_EXPT_BUFS", "12"))
SCP_BUFS = int(_os.environ.get("KBASS_SCP_BUFS", "2"))
DVE_FRAC = float(_os.environ.get("KBASS_DVE_FRAC", "0.5"))
EXACT_RECIP = bool(int(_os.environ.get("KBASS_EXACT_RECIP", "0")))
OSB_DMA = bool(int(_os.environ.get("KBASS_OSB_DMA", "0")))
NORM_SPREAD = bool(int(_os.environ.get("KBASS_NORM_SPREAD", "1")))
# fp8 DoubleRow scores: q/k are pre-scaled x8 on the host, converted to
# fp8e4 on device, and the score matmuls run in DoubleRow mode (0.5
# cycles/col, contraction 16x2).  The x64 score scale is folded into the
# exp: ACT activation scale=1/64, DVE poly s0 = PC_S/64.
FP8_QK = bool(int(_os.environ.get("KBASS_FP8", "1")))
QK_PRE = 8.0 if FP8_QK else 1.0
F8 = mybir.dt.float8e4
# timing-only diagnostics (break correctness):
NO_CTXDEN = bool(int(_os.environ.get("KBASS_NO_CTXDEN", "0")))
NO_EXP = bool(int(_os.environ.get("KBASS_NO_EXP", "0")))
N_CORES = 8

# ---------------------------------------------------------------------------
# EXP_POLY_ANT: exp(x) ~ ((x*s + t)^2 + v)^2 as ONE custom DVE instruction.
# Registered into concourse.dve_ops.OPS at import (the documented extension
# point; the staged package is read-only so the append happens here).
import concourse.dve_ops as _dve_ops
from concourse.dve_spec import (Spec as _Spec, Src0 as _Src0, C0 as _C0,
                                C1 as _C1, C2 as _C2, sq as _sq,
                                lower as _lower, _has_src1)
from concourse.dve_uop import DveOpSpec as _DveOpSpec

_QA = 0.12390159539006088                  # minimax quad of e^{x/2}: a x^2+b x+c
_QB = 0.12390159539006088 * 4.105499341616261
_QC = 1.0006054755744616
PC_S = float(np.sqrt(_QA))
PC_T = float(_QB / (2 * np.sqrt(_QA)))
PC_V = float(_QC - PC_T * PC_T)


def _exp_ref(in0, in1, s0, s1, imm2):
    x = in0.astype(np.float32)
    q = (x * s0 + s1) ** 2 + imm2
    return (q * q).astype(np.float32)


def _register_exp_op():
    name = "EXP_POLY_ANT"
    if name in _dve_ops._SUB_OPCODE_FOR_NAME:
        for op in _dve_ops.OPS:
            if op.name == name:
                return op
    spec = _Spec(body=_sq(_sq(_Src0 * _C0 + _C1) + _C2), reference=_exp_ref)
    shas = {}
    for ver in ("v3", "v4"):
        uops = _lower(spec, ver=ver)
        shas[ver] = _DveOpSpec(name=name, opcode=31, uops=uops,
                               rd1_en=_has_src1(spec)).sha(ver)
    op = _dve_ops.DveOp(name, spec, subdim=False, uops_sha=shas)
    _dve_ops.OPS.append(op)
    _dve_ops._SUB_OPCODE_FOR_NAME[name] = (
        _dve_ops._CUSTOM_DVE_ROW_BASE + len(_dve_ops.OPS) - 1)
    _dve_ops.CUSTOM_DVE_SPECS[name] = spec
    return op


EXP_POLY_ANT = _register_exp_op()

_cached = {}


def _build_nc(reps=1):
    nc = bacc.Bacc("TRN2", target_bir_lowering=False, debug=False,
                   num_devices=N_CORES)

    hidden = nc.declare_dram_parameter("hidden", [S, D], F32, isOutput=False).ap()
    pos = nc.declare_dram_parameter("pos", [S, D], F32, isOutput=False).ap()
    wq = nc.declare_dram_parameter("wq", [D, HPG * HD], F16, isOutput=False).ap()
    wk = nc.declare_dram_parameter("wk", [D, HPG * HD], F16, isOutput=False).ap()
    wv = nc.declare_dram_parameter("wv", [D, HPG * HD], F16, isOutput=False).ap()
    wo = nc.declare_dram_parameter("wo", [HPG * HD, D], F16, isOutput=False).ap()
    out = nc.declare_dram_parameter("out", [S, D], F32, isOutput=True).ap()

    with tile.TileContext(nc) as tc, ExitStack() as stack:
        # ---- persistent SBUF ----
        pers = stack.enter_context(tc.tile_pool(name="persist", bufs=1))
        wq_sb = pers.tile([128, DT, HPG * HD], F16, name="wq_sb")
        wk_sb = pers.tile([128, DT, HPG * HD], F16, name="wk_sb")
        wv_sb = pers.tile([128, DT, HPG * HD], F16, name="wv_sb")
        wo_sb = pers.tile([128, D], F16, name="wo_sb")
        ident = pers.tile([128, 128], F32, name="ident")
        hidT = pers.tile([128, DT, S], F16, name="hidT")
        posT = pers.tile([128, DT, S], F16, name="posT")
        qT = pers.tile([128, S], F16, name="qT")
        kT = pers.tile([128, S], F16, name="kT")
        if FP8_QK:
            # fp8 copies for DoubleRow score matmuls: slot j holds head-dim
            # subtile e in [16j, 16j+16) at partitions 32h..32h+15.
            qT8 = pers.tile([128, 2, S], F8, name="qT8")
            kT8 = pers.tile([128, 2, S], F8, name="kT8")
        # v for the merged ctx+den matmul, one 96-wide slab per (m, half):
        # cols [v_h_even (0:32) | ones (32) | zeros (33:64) | v_h_odd (64:96)].
        # The zero pad keeps v_h_odd's output rows at partition base 64
        # (aligned partition window for the norm muls); the single ones
        # column yields BOTH heads' denominators (row 32, per-head q cols).
        vstack = pers.tile([128, NT, 2, 96], F16, name="vstack")

        for dt in range(DT):
            nc.sync.dma_start(out=wq_sb[:, dt, :], in_=wq[ts(dt, 128), :])
            nc.sync.dma_start(out=wk_sb[:, dt, :], in_=wk[ts(dt, 128), :])
            nc.sync.dma_start(out=wv_sb[:, dt, :], in_=wv[ts(dt, 128), :])
        nc.sync.dma_start(out=wo_sb, in_=wo)
        make_identity(nc, ident)
        nc.vector.memset(vstack, 0.0)
        nc.vector.memset(vstack[:, :, :, ds(32, 1)], 1.0)

        # ---- prep: per 512-row group g, transpose hidden/pos into [d, s]
        # layouts, then project that group's v / kT / qT slices
        # (kT/qT accumulate Wx^T hidT + Wx^T posT; hp is never formed).
        with tc.tile_pool(name="tr_psum", bufs=1, space="PSUM") as trp, \
             tc.tile_pool(name="pj_psum", bufs=1, space="PSUM") as pjp, \
             tc.tile_pool(name="io", bufs=4) as io:
            for g in range(NT // 4):
                tr_h = [trp.tile([128, 512], F32, name=f"tr_h{dt}")
                        for dt in range(DT)]
                tr_p = [trp.tile([128, 512], F32, name=f"tr_p{dt}")
                        for dt in range(DT)]
                for j in range(4):
                    m = 4 * g + j
                    hid_t = io.tile([128, D], F32, name="hid_t")
                    nc.sync.dma_start(out=hid_t, in_=hidden[ts(m, 128), :])
                    pos_t = io.tile([128, D], F32, name="pos_t")
                    nc.gpsimd.dma_start(out=pos_t, in_=pos[ts(m, 128), :])
                    for dt in range(DT):
                        nc.tensor.transpose(tr_h[dt][:, ts(j, 128)],
                                            hid_t[:, ts(dt, 128)], ident)
                        nc.tensor.transpose(tr_p[dt][:, ts(j, 128)],
                                            pos_t[:, ts(dt, 128)], ident)
                nc.vector.tensor_copy(hidT[:, 0, ts(g, 512)], tr_h[0])
                nc.scalar.copy(hidT[:, 1, ts(g, 512)], tr_h[1])
                nc.vector.tensor_copy(posT[:, 0, ts(g, 512)], tr_p[0])
                nc.scalar.copy(posT[:, 1, ts(g, 512)], tr_p[1])

                ps_v = pjp.tile([128, 512], F32, name="ps_v")
                for j in range(4):
                    m = 4 * g + j
                    for dt in range(DT):
                        nc.tensor.matmul(ps_v[:, ts(j, 128)],
                                         lhsT=hidT[:, dt, ts(m, 128)],
                                         rhs=wv_sb[:, dt, :],
                                         start=(dt == 0), stop=(dt == DT - 1))
                pv = ps_v.rearrange("p (m hf jc) -> p m hf jc", m=4, hf=2)
                for j in range(2):
                    nc.vector.tensor_copy(
                        vstack[:, 4 * g:4 * g + 4, :, ds(64 * j, HD)],
                        pv[:, :, :, ds(32 * j, HD)])

                for (w_sb, dest, pname) in ((wk_sb, kT, "ps_k"),
                                            (wq_sb, qT, "ps_q")):
                    ps_qk = pjp.tile([128, 512], F32, name=pname)
                    first = True
                    for dt in range(DT):
                        for src in (hidT, posT):
                            nc.tensor.matmul(ps_qk,
                                             lhsT=w_sb[:, dt, :],
                                             rhs=src[:, dt, ts(g, 512)],
                                             start=first,
                                             stop=(dt == DT - 1 and src is posT))
                            first = False
                    if w_sb is wk_sb:
                        nc.scalar.copy(dest[:, ts(g, 512)], ps_qk)
                    else:
                        nc.vector.tensor_copy(dest[:, ts(g, 512)], ps_qk)

            if FP8_QK:
                shuf = [16 + (i % 16) for i in range(32)]
                for (src, dst8) in ((qT, qT8), (kT, kT8)):
                    nc.vector.tensor_copy(dst8[:, 0, :], src)
                    tmp = io.tile([128, S], F16, name="cvt_t")
                    nc.vector.stream_shuffle(tmp, src, shuf)
                    nc.vector.tensor_copy(dst8[:, 1, :], tmp)

        # ---- main attention loop ----
        with tc.tile_pool(name="sc_psum", bufs=SCP_BUFS, space="PSUM") as scp, \
             tc.tile_pool(name="cd_psum", bufs=1, space="PSUM") as cdp, \
             tc.tile_pool(name="out_psum", bufs=2, space="PSUM") as outp, \
             tc.tile_pool(name="expt_sb", bufs=EXPT_BUFS) as exps, \
             tc.tile_pool(name="tail_sb", bufs=2) as tls, \
             tc.tile_pool(name="ctxn_sb", bufs=2) as ctxns, \
             tc.tile_pool(name="osb_sb", bufs=2) as osbs:
          # One persistent ctx+den accumulator [128, 1024] (2 banks), shared
          # by the two head-pair halves back-to-back (half-major loop): rows
          # 0-31 ctx of h_even (q cols 0-511), row 32 both dens (per-head q
          # cols), rows 64-95 ctx of h_odd (q cols 512-1023).  Junk rows are
          # memset once; live rows reset each half by start=True.
          ctxden = cdp.tile([128, 1024], F32, name="ctxden")
          nc.vector.memset(ctxden, 1.0)
          # Per-half persistent den staging tiles (rows {0,32} = the two
          # dens; junk rows memset 1.0 once).  Custom-DVE ops need base
          # partition 0, hence per-half tiles instead of one [128,512].
          dsb = [pers.tile([64, 512], F32, name=f"dsb{i}") for i in range(2)]
          for t in dsb:
              nc.vector.memset(t, 1.0)

          def _main_body(_iv=None):
            state = {"ctxn": None, "gchunk": 0, "dve_acc": 0.0, "hold": 0}
            pend = []      # (n, half, m, expt)
            tail2 = []     # (due_gchunk, emit_fn)
            dveq = []      # deferred DVE norm ops, drained 1/chunk

            def _emit_norm(n, half):
                # dens staged in dsb[half] rows {0,32}: one approx-recip +
                # one shuffle, then per-head muls into the fp16 ctxn tile.
                # Queued and drained one op per chunk; ctxden pops pause
                # (hold) so the muls are emitted before the next half's
                # start=True matmul (emission order defines dependencies).
                rc = tls.tile([64, 512], F32, name="rc")
                rb = tls.tile([64, 512], F32, name="rb")
                if half == 0:
                    state["ctxn"] = ctxns.tile([128, 512], F16, name="ctxn")
                ctxn = state["ctxn"]
                dveq.append(
                    lambda: nc.vector.reciprocal_approx_fast(rc, dsb[half]))
                dveq.append(lambda: nc.vector.stream_shuffle(rb, rc, [0] * 32))
                for j in range(2):
                    h = 2 * half + j
                    dveq.append(lambda h=h, j=j: nc.vector.tensor_mul(
                        ctxn[ds(32 * h, 32), :],
                        ctxden[ds(64 * j, 32), ts(j, 512)],
                        rb[ds(32 * j, 32), :]))
                if half == 1:
                    tail2.append((state["gchunk"] + 8,
                                  lambda: _emit_tail2(n, ctxn)))
                state["hold"] = len(dveq)
                if not NORM_SPREAD:
                    while dveq:
                        dveq.pop(0)()
                    state["hold"] = 0

            def _emit_tail2(n, ctxn):
                for t in range(2):
                    ps_out = outp.tile([128, 512], F32, name="ps_out")
                    for u in range(2):
                        nc.tensor.matmul(ps_out[:, ts(u, 256)],
                                         lhsT=ctxn[:, ts(2 * t + u, 128)],
                                         rhs=wo_sb, start=True, stop=True)
                    dst = out[ds(512 * n + 256 * t, 256), :].rearrange(
                        "(u p) d -> p u d", u=2)
                    if OSB_DMA:
                        nc.sync.dma_start(
                            out=dst,
                            in_=ps_out.rearrange("p (u d) -> p u d", u=2))
                    else:
                        osb = osbs.tile([128, 512], F32, name="osb")
                        nc.vector.tensor_copy(osb, ps_out)
                        nc.sync.dma_start(
                            out=dst,
                            in_=osb.rearrange("p (u d) -> p u d", u=2))

            def _ctx_den(n, half, m, expt):
                if NO_CTXDEN:
                    return
                for j in range(2):
                    nc.tensor.matmul(
                        ctxden[ds(0, 96), ts(j, 512)],
                        lhsT=vstack[:, m, half, :],
                        rhs=expt[:, ts(j, 512)],
                        start=(m == 0), stop=(m == NT - 1),
                        tile_position=(0, 0),
                        skip_group_check=True)
                if m == NT - 1:
                    for j in range(2):
                        nc.scalar.copy(dsb[half][ds(32 * j, 1), :],
                                       ctxden[ds(32, 1), ts(j, 512)])
                    _emit_norm(n, half)

            for n in range(NB):
                for half in range(2):
                    for m in range(NT):
                        ps_sc = scp.tile([128, 1024], F32, name="ps_sc")
                        for j in range(2):
                            h = 2 * half + j
                            if FP8_QK:
                                nc.tensor.matmul(
                                    ps_sc[:, ts(j, 512)],
                                    lhsT=kT8[ds(32 * h, 16), :, ts(m, 128)],
                                    rhs=qT8[ds(32 * h, 16), :, ts(n, 512)],
                                    start=True, stop=True,
                                    perf_mode=mybir.MatmulPerfMode.DoubleRow,
                                    tile_position=(32 * h, 0))
                            else:
                                nc.tensor.matmul(
                                    ps_sc[:, ts(j, 512)],
                                    lhsT=kT[ds(32 * h, 32), ts(m, 128)],
                                    rhs=qT[ds(32 * h, 32), ts(n, 512)],
                                    start=True, stop=True,
                                    tile_position=(32 * h, 0))
                        expt = exps.tile([128, 1024], F16, name="expt")
                        state["dve_acc"] += DVE_FRAC
                        if NO_EXP:
                            if state["gchunk"] < EXPT_BUFS:
                                nc.vector.memset(expt, 0.5)
                            state["dve_acc"] = 0.0
                        elif state["dve_acc"] >= 1.0:
                            state["dve_acc"] -= 1.0
                            nc.vector._custom_dve(
                                EXP_POLY_ANT, out=expt, in0=ps_sc,
                                s0=PC_S / (QK_PRE * QK_PRE),
                                s1=PC_T, imm2=PC_V)
                        else:
                            nc.scalar.activation(
                                expt, ps_sc,
                                mybir.ActivationFunctionType.Exp,
                                scale=1.0 / (QK_PRE * QK_PRE))
                        if dveq:
                            dveq.pop(0)()
                        pend.append((n, half, m, expt))
                        if state["hold"] > 0:
                            state["hold"] -= 1
                        else:
                            pops = 0
                            while len(pend) > PEND_DEPTH and pops < 2:
                                _ctx_den(*pend.pop(0))
                                pops += 1
                        state["gchunk"] += 1
                        while tail2 and tail2[0][0] <= state["gchunk"]:
                            tail2.pop(0)[1]()
            while pend:
                _ctx_den(*pend.pop(0))
            while dveq:
                dveq.pop(0)()
            while tail2:
                tail2.pop(0)[1]()
          if reps == 1:
              _main_body()
          else:
              with tc.For_i(0, reps, 1) as iv:
                  _main_body(iv)
    nc.compile()
    return nc


def _get_nc(reps=1):
    key = f"nc{reps}"
    if key not in _cached:
        _cached[key] = _build_nc(reps)
    return _cached[key]


def make_in_maps(hidden_states, position_embeddings, Wq, Wk, Wv, Wo):
    """Per-core input dict for run_bass_kernel_spmd (fp16 weights,
    SCALING folded into Wq)."""
    wq16 = (Wq.reshape(D, H * HD) * (SCALING * QK_PRE)).astype(NP16)
    wk16 = (Wk.reshape(D, H * HD) * QK_PRE).astype(NP16)
    wv16 = Wv.reshape(D, H * HD).astype(NP16)
    wo16 = Wo.astype(NP16)
    in_maps = []
    for c in range(N_CORES):
        b, hg = c % B, c // B
        cs = slice(hg * HPG * HD, (hg + 1) * HPG * HD)
        in_maps.append({
            "hidden": np.ascontiguousarray(hidden_states[b]),
            "pos": np.ascontiguousarray(position_embeddings[b]),
            "wq": np.ascontiguousarray(wq16[:, cs]),
            "wk": np.ascontiguousarray(wk16[:, cs]),
            "wv": np.ascontiguousarray(wv16[:, cs]),
            "wo": np.ascontiguousarray(wo16[cs, :]),
        })
    return in_maps


def _reference_numpy(hidden_states, position_embeddings, attention_mask,
                     Wq, bq, Wk, bk, Wv, bv, Wo, bo):
    # Fallback for nonzero mask/bias (never hit for this problem's spec).
    hp = hidden_states + position_embeddings
    q = np.einsum("bsd,dhe->bshe", hp, Wq) + bq
    k = np.einsum("bsd,dhe->bshe", hp, Wk) + bk
    v = np.einsum("bsd,dhe->bshe", hidden_states, Wv) + bv
    q = q * SCALING
    scores = np.einsum("bqhe,bkhe->bhqk", q, k) + attention_mask[:, None]
    scores -= scores.max(axis=-1, keepdims=True)
    e = np.exp(scores)
    attn = e / e.sum(axis=-1, keepdims=True)
    ctx = np.einsum("bhqk,bkhe->bqhe", attn, v).reshape(B, S, D)
    return (np.einsum("bsd,de->bse", ctx, Wo) + bo).astype(np.float32)


def kernel(hidden_states, position_embeddings, attention_mask,
           Wq, bq, Wk, bk, Wv, bv, Wo, bo, _want_results=False,
           _trace=False, _tmpdir=None):
    args = [np.asarray(a, dtype=np.float32) for a in
            (hidden_states, position_embeddings, attention_mask,
             Wq, bq, Wk, bk, Wv, bv, Wo, bo)]
    (hidden_states, position_embeddings, attention_mask,
     Wq, bq, Wk, bk, Wv, bv, Wo, bo) = args

    if (np.any(attention_mask) or np.any(bq) or np.any(bk) or np.any(bv)):
        return _reference_numpy(hidden_states, position_embeddings,
                                attention_mask, Wq, bq, Wk, bk, Wv, bv, Wo, bo)

    nc = _get_nc()
    in_maps = make_in_maps(hidden_states, position_embeddings, Wq, Wk, Wv, Wo)
    res = bass_utils.run_bass_kernel_spmd(nc, in_maps, list(range(N_CORES)),
                                          trace=_trace, tmpdir=_tmpdir)
    out = np.empty((B, S, D), np.float32)
    for b in range(B):
        out[b] = res.results[b]["out"] + res.results[b + B]["out"] + bo
    if _want_results:
        return out, res
    return out
